# revision 2
# baseline (speedup 1.0000x reference)
"""Trainium2 Bass kernel for a supervised-contrastive-style loss (fp8 version).

Reference computation:
  - dropout(p=0.5, scale 2, jax key 42) on gathered class-2 rows, concat -> feats [N2, D]
  - fn = feats / max(||feats||, 1e-8);  sim = fn @ fn.T / T
  - denom_i = sum_j exp(sim_ij) * [labs_i == labs_j]
  - loss = -mean(sim_ii - log denom_i)

Strategy (v2, evolved from the f32r baseline):
  * Host mirrors the reference prologue bit-exactly, sorts rows by class so the
    label mask is block-diagonal, and quantizes fn*ALPHA to fp8e4m3.
  * Device computes only same-class upper-triangle (row-tile[128] x
    col-panel[512]) blocks with fp8 DoubleRow matmuls (K=256 per instruction,
    2 MACs/cell/cycle).  All column panels stay resident in SBUF (fp8), so
    steady-state iterations do zero input DMA.
  * Work is slot-outer: core k owns row tiles k, k+8, ...; slot j covers
    panels p in [2j, P).  Per slot the 4 K-chunks reuse the same stationary
    operand across the slot's panel chunk (weight reuse) and a dedicated
    [128,128] diag matmul (same operands/accumulation order as the panel job)
    yields raw sim_ii so log(denom) - sim_ii cancels structurally.
  * Row sums via ScalarE exp accum_out; strictly-upper column sums via
    ones-vector matmul, DMA'd straight from PSUM.
  * Host: float64 combination, dead-row/pad corrections via an exp(0) witness.
"""

import math

import numpy as np
import ml_dtypes

TEMPERATURE = 0.07
DROP_P = 0.5
EPS = 1e-8
NCORES = 8
KP = 128     # partition size
PANEL = 512  # max matmul moving free dim (one PSUM bank of fp32)
KT2 = 4      # number of K=256 DoubleRow chunks over D=1024
ALPHA = 16.0  # fp8 pre-scale on fn
F8 = ml_dtypes.float8_e4m3

_CACHE = {}


# --------------------------------------------------------------------------
# host-side preparation
# --------------------------------------------------------------------------

def _host_prep(features, labels, aug_indices):
    """Mirror the reference's prologue op-for-op on the default jax backend so
    the dropout PRNG bits and fn values match the graded reference exactly."""
    import jax
    import jax.numpy as jnp

    features = jnp.asarray(np.asarray(features))
    labels_np = np.asarray(labels)
    aug_np = np.asarray(aug_indices)

    pert = features[jnp.asarray(aug_np)]
    keep = jax.random.bernoulli(jax.random.key(42), 1.0 - DROP_P, pert.shape)
    pert = jnp.where(keep, pert * 2.0, jnp.zeros((), dtype=pert.dtype))
    feats = jnp.concatenate([features, pert], axis=0)

    norms = jnp.sqrt(jnp.sum(feats * feats, axis=1, keepdims=True))
    fn = np.asarray(feats / jnp.maximum(norms, EPS)).astype(np.float32)
    labs = np.concatenate([labels_np, labels_np[aug_np]], axis=0)

    perm = np.argsort(labs, kind="stable")
    fn_sorted = np.ascontiguousarray(fn[perm])
    labs_sorted = labs[perm]
    return fn, labs, perm, fn_sorted, labs_sorted


class _Plan:
    """Compile-time structure shared by program builder, simulator, finisher.

    Per class c (counts in sorted-label order):
      RT[c] global 128-row tiles; R[c] = ceil(RT/8) per-core slots
      P[c]  column panels; widths W (last panel rounded to even)
    Core k's slot (c, j) holds physical tile t = k + 8*j (dead if t >= RT).
    Job set: per slot (c, j), panels p in [2j, P[c]) -- covers p >= t//4 for
    both tile groups t//4 in {2j, 2j+1}; extras are ignored on host.
    """

    def __init__(self, n2, d, class_counts):
        assert d == KT2 * 256
        self.n2 = n2
        self.d = d
        self.counts = list(class_counts)
        self.ncls = len(self.counts)
        self.RT = [math.ceil(c / KP) for c in self.counts]
        self.R = [math.ceil(rt / NCORES) for rt in self.RT]
        self.P = [math.ceil(c / PANEL) for c in self.counts]
        self.Wreal = [c - (p - 1) * PANEL for c, p in zip(self.counts, self.P)]
        self.W = [w + (w & 1) for w in self.Wreal]
        self.row_slots = sum(self.R) * KP
        self.col_slots = sum(p * PANEL for p in self.P)
        self.cls_row_off = np.cumsum([0] + self.counts).tolist()
        self.panel_off = np.cumsum([0] + [p * PANEL for p in self.P]).tolist()

        # flat slots, big (most panels) first
        slots = [(c, j) for c in range(self.ncls) for j in range(self.R[c])]
        slots.sort(key=lambda cj: -(self.P[cj[0]] - 2 * cj[1]))
        self.slots = slots
        self.nslots = len(slots)
        self.slot_index = {cj: i for i, cj in enumerate(slots)}

        # jobs in slot-outer order; per-slot chunks: [2j] alone, rest <= 3.
        # One fused exp + one partials column per chunk.
        self.jobs = []
        self.chunks = []     # (chunk_id, si, c, j, [panel...]) emission units
        for si, (c, j) in enumerate(slots):
            panels = list(range(2 * j, self.P[c]))
            ch = [[panels[0]]]
            rest = panels[1:]
            for i in range(0, len(rest), 2):
                ch.append(rest[i:i + 2])
            for cl in ch:
                self.chunks.append((len(self.chunks), si, c, j, cl))
            for p in panels:
                self.jobs.append((c, p, j, self.width(c, p)))
        self.nchunks = len(self.chunks)
        self.chunk_id = {}   # (c, j) -> [chunk ids in panel order]
        for cid, si, c, j, cl in self.chunks:
            self.chunk_id.setdefault((c, j), []).append(cid)
        self.njobs = len(self.jobs)
        self.job_id = {(c, p, j): i for i, (c, p, j, w) in
                       enumerate(self.jobs)}

    def width(self, c, p):
        return PANEL if p < self.P[c] - 1 else self.W[c]

    def phys_tile(self, core, j):
        return core + NCORES * j

    def realrows(self, c, t):
        return int(min(max(self.counts[c] - KP * t, 0), KP))


def _build_host_arrays(plan, fn_sorted):
    """fp8 cols tensor (shared) and per-core fp8 lhsT tensors.

    Layout: [KT2, KP, 2, slots] so SBUF tile [KP, KT2, 2, slots] loads with
    4 large DMAs.  Element (k2, i, h, s) = fnT8[(2*k2 + h)*128 + i, s].
    """
    fnT8 = (fn_sorted.T * np.float32(ALPHA)).astype(F8)   # [D, N2]
    d, n2 = fnT8.shape

    cols = np.zeros((KT2, KP, 2, plan.col_slots), dtype=F8)
    src = fnT8.reshape(KT2, 2, KP, n2)
    for c in range(plan.ncls):
        cnt = plan.counts[c]
        cols[:, :, :, plan.panel_off[c]: plan.panel_off[c] + cnt] = (
            src[:, :, :, plan.cls_row_off[c]: plan.cls_row_off[c] + cnt]
            .transpose(0, 2, 1, 3))

    lhsTs = []
    for core in range(NCORES):
        lt = np.zeros((KT2, KP, 2, plan.row_slots), dtype=F8)
        for si, (c, j) in enumerate(plan.slots):
            t = plan.phys_tile(core, j)
            if t >= plan.RT[c]:
                continue
            nreal = plan.realrows(c, t)
            g0 = plan.cls_row_off[c] + KP * t
            lt[:, :, :, si * KP: si * KP + nreal] = (
                src[:, :, :, g0: g0 + nreal].transpose(0, 2, 1, 3))
        lhsTs.append(lt)
    return cols, lhsTs


def _scale32():
    return np.float32(1.0) / (np.float32(ALPHA * ALPHA)
                              * np.float32(TEMPERATURE))


# --------------------------------------------------------------------------
# bass program
# --------------------------------------------------------------------------

def _build_program(plan, reps=1):
    import concourse.bacc as bacc
    import concourse.tile as tile
    import concourse.mybir as mybir

    f32 = mybir.dt.float32
    f32r = mybir.dt.float32r
    f8 = mybir.dt.float8e4
    DR = mybir.MatmulPerfMode.DoubleRow
    sc = float(_scale32())

    CHUNK = 2
    nc = bacc.Bacc("TRN2", target_bir_lowering=False, debug=False)
    lhsT_d = nc.dram_tensor("lhsT", [KT2, KP, 2, plan.row_slots], f8,
                            kind="ExternalInput")
    cols_d = nc.dram_tensor("cols", [KT2, KP, 2, plan.col_slots], f8,
                            kind="ExternalInput")
    ident_d = nc.dram_tensor("ident", [KP, KP], f32, kind="ExternalInput")
    ones_d = nc.dram_tensor("ones", [KP, 1], f32r, kind="ExternalInput")
    part_d = nc.dram_tensor("partials", [KP, plan.nchunks], f32,
                            kind="ExternalOutput")
    diag_d = nc.dram_tensor("diag", [KP, plan.nslots], f32,
                            kind="ExternalOutput")
    csum_d = nc.dram_tensor("csum", [plan.njobs, PANEL], f32,
                            kind="ExternalOutput")
    e0_d = nc.dram_tensor("e0", [KP, 1], f32, kind="ExternalOutput")

    with tile.TileContext(nc) as tc:
        with (
            tc.tile_pool(name="persist", bufs=1) as persist,
            tc.tile_pool(name="work", bufs=3) as work,
            tc.tile_pool(name="psum", bufs=3, space="PSUM") as psum_main,
            tc.tile_pool(name="psumc", bufs=2, space="PSUM") as psum_cs,
        ):
            colsT = persist.tile([KP, KT2, 2, plan.col_slots], f8)
            lhsT = persist.tile([KP, KT2, 2, plan.row_slots], f8)
            for k2 in range(KT2):
                nc.sync.dma_start(out=colsT[:, k2], in_=cols_d[k2])
                nc.sync.dma_start(out=lhsT[:, k2], in_=lhsT_d[k2])
            ident = persist.tile([KP, KP], f32)
            nc.sync.dma_start(out=ident, in_=ident_d[:])
            ones = persist.tile([KP, 1], f32r)
            nc.sync.dma_start(out=ones, in_=ones_d[:])
            partials = persist.tile([KP, plan.nchunks], f32)
            diag = persist.tile([KP, plan.nslots], f32)

            # exp(0) witness (dead-row / pad correction on host)
            zt = persist.tile([KP, 1], f32)
            nc.vector.memset(zt, 0.0)
            e0t = persist.tile([KP, 1], f32)
            nc.scalar.activation(out=e0t, in_=zt,
                                 func=mybir.ActivationFunctionType.Exp,
                                 scale=sc)
            nc.sync.dma_start(out=e0_d[:], in_=e0t)

            def emit_body():
                # Colsum matmuls for chunk i are emitted after chunk i+1's
                # main matmuls (they depend on chunk i's ScalarE exp and the
                # in-order PE would stall on them otherwise).  Four colsum
                # rows land in one [97, PANEL] PSUM tile at partitions
                # {0,32,64,96} via tile_position, so one DVE copy and one
                # (partition-strided) DMA move four jobs at once.
                pending = []          # delayed colsum work: (e_ap, w, jid)
                CSB = 8
                batch = {"tile": None, "base": None, "n": 0, "wlast": PANEL}

                def flush_batch():
                    n, b0, wl = batch["n"], batch["base"], batch["wlast"]
                    if n:
                        full = n if wl == PANEL else n - 1
                        if full:
                            nc.sync.dma_start(
                                out=csum_d[b0:b0 + full, :],
                                in_=batch["tile"][0:1, :full * PANEL])
                        if wl < PANEL:
                            nc.sync.dma_start(
                                out=csum_d[b0 + full:b0 + n, :wl],
                                in_=batch["tile"][
                                    0:1, full * PANEL: full * PANEL + wl])
                    batch["tile"] = None
                    batch["n"] = 0

                def flush_pending():
                    for e_ap, w_, jid_ in pending:
                        pcs = psum_cs.tile([1, PANEL], f32, name="pcs")
                        nc.tensor.matmul(pcs[:, :w_], ones, e_ap[:, :w_],
                                         start=True, stop=True)
                        if (batch["tile"] is not None
                                and jid_ != batch["base"] + batch["n"]):
                            flush_batch()
                        if batch["tile"] is None:
                            batch["tile"] = work.tile([1, CSB * PANEL], f32,
                                                      tag="csb", name="csb",
                                                      bufs=2)
                            batch["base"] = jid_
                        idx = jid_ - batch["base"]
                        nc.vector.tensor_copy(
                            batch["tile"][0:1,
                                          idx * PANEL: idx * PANEL + w_],
                            pcs[:, :w_])
                        batch["n"] = idx + 1
                        batch["wlast"] = w_
                        if batch["n"] == CSB or w_ < PANEL:
                            flush_batch()
                    pending.clear()

                for cid, si, c, j, chunk in plan.chunks:
                    rsl = slice(si * KP, si * KP + KP)
                    first = (chunk[0] == 2 * j)
                    big = psum_main.tile([KP, CHUNK * PANEL], f32, name="ps")
                    for k2 in range(KT2):
                        w_ap = lhsT[:, k2, :, rsl]
                        for pi, p in enumerate(chunk):
                            w = plan.width(c, p)
                            c0 = plan.panel_off[c] + p * PANEL
                            nc.tensor.matmul(
                                big[:, pi * PANEL: pi * PANEL + w], w_ap,
                                colsT[:, k2, :, c0:c0 + w],
                                start=(k2 == 0), stop=(k2 == KT2 - 1),
                                perf_mode=DR)
                        if first:
                            nc.tensor.matmul(
                                big[:, PANEL:PANEL + KP], w_ap,
                                lhsT[:, k2, :, rsl],
                                start=(k2 == 0), stop=(k2 == KT2 - 1),
                                perf_mode=DR)
                    flush_pending()
                    tot = (len(chunk) - 1) * PANEL + plan.width(c, chunk[-1])
                    e = work.tile([KP, CHUNK * PANEL], f32r, tag="etile",
                                  name="e", bufs=3)
                    nc.scalar.activation(
                        out=e[:, :tot], in_=big[:, :tot],
                        func=mybir.ActivationFunctionType.Exp,
                        scale=sc,
                        accum_out=partials[:, cid:cid + 1])
                    for pi, p in enumerate(chunk):
                        if p != 2 * j:
                            pending.append(
                                (e[:, pi * PANEL:(pi + 1) * PANEL],
                                 plan.width(c, p), plan.job_id[(c, p, j)]))
                    if first:
                        tmp = work.tile([KP, KP], f32, tag="dtmp",
                                        name="dtmp")
                        nc.vector.tensor_mul(tmp, big[:, PANEL:PANEL + KP],
                                             ident)
                        nc.vector.reduce_sum(diag[:, si:si + 1], tmp,
                                             axis=mybir.AxisListType.X)
                flush_pending()
                flush_batch()

            if reps > 1:
                with tc.For_i(0, reps, 1):
                    emit_body()
            else:
                emit_body()

            nc.sync.dma_start(out=part_d[:], in_=partials)
            nc.sync.dma_start(out=diag_d[:], in_=diag)
    nc.compile()
    return nc


# --------------------------------------------------------------------------
# numpy simulation of the device outputs (for logic validation)
# --------------------------------------------------------------------------

def _simulate_device(plan, cols, lhsTs):
    sc = _scale32()
    results = []
    colsf = (cols.astype(np.float32)
             .transpose(0, 2, 1, 3).reshape(plan.d, plan.col_slots))
    for core in range(NCORES):
        lf = (lhsTs[core].astype(np.float32)
              .transpose(0, 2, 1, 3).reshape(plan.d, plan.row_slots))
        partials = np.zeros((KP, plan.nchunks), dtype=np.float32)
        diag = np.zeros((KP, plan.nslots), dtype=np.float32)
        csum = np.zeros((plan.njobs, PANEL), dtype=np.float32)
        for cid, si, c, j, chunk in plan.chunks:
            W = lf[:, si * KP: si * KP + KP]
            for p in chunk:
                w = plan.width(c, p)
                jid = plan.job_id[(c, p, j)]
                c0 = plan.panel_off[c] + p * PANEL
                s = (W.T @ colsf[:, c0:c0 + w]).astype(np.float32)
                e = np.exp((s * sc).astype(np.float32), dtype=np.float32)
                partials[:, cid] += e.sum(axis=1, dtype=np.float32)
                if p != 2 * j:
                    csum[jid, :w] = e.sum(axis=0, dtype=np.float32)
        for si, (c, j) in enumerate(plan.slots):
            W = lf[:, si * KP: si * KP + KP]
            sd = (W.T @ W).astype(np.float32)
            diag[:, si] = np.diagonal(sd)
        results.append({"partials": partials, "diag": diag, "csum": csum,
                        "e0": np.ones((KP, 1), dtype=np.float32)})
    return results


# --------------------------------------------------------------------------
# host-side finish
# --------------------------------------------------------------------------

def _finish(plan, results):
    """Combine per-core device outputs into the scalar loss (float64).

    Row i (class c, class-row g = 128*t + i, owner core k = t%8, j = t//8):
      denom_g = sum over jobs (c,p,j), p >= t//4 of partials[i, job]   (rows)
              + sum over tiles t' with t'//4 < p_g of
                    csum[job(c, p_g, j'), g - 512*p_g] - dead(t')*e0   (cols)
      x_g     = f32(diag[i, slot(c, t//8)] * sc)
      loss_g  = log(denom_g) - x_g
    """
    sc = _scale32()
    total = 0.0
    nrows = 0
    for c in range(plan.ncls):
        cnt = plan.counts[c]
        denom = np.zeros(cnt, dtype=np.float64)
        x = np.zeros(cnt, dtype=np.float64)
        for core in range(NCORES):
            partials = results[core]["partials"].astype(np.float64)
            diag = results[core]["diag"]
            csum = results[core]["csum"].astype(np.float64)
            e0 = float(results[core]["e0"][0, 0])
            for j in range(plan.R[c]):
                t = plan.phys_tile(core, j)
                if t >= plan.RT[c]:
                    continue
                si = plan.slot_index[(c, j)]
                m = plan.realrows(c, t)
                rows = slice(KP * t, KP * t + m)
                # chunk 0 holds panel 2j alone; skip it when the tile's
                # diagonal panel is 2j+1
                cids = plan.chunk_id[(c, j)]
                use = cids if t // 4 == 2 * j else cids[1:]
                for cid in use:
                    denom[rows] += partials[:m, cid]
                denom[rows] -= (plan.W[c] - plan.Wreal[c]) * e0
                for p in range(t // 4 + 1, plan.P[c]):
                    jid = plan.job_id[(c, p, j)]
                    wr = min(plan.jobs[jid][3], plan.counts[c] - PANEL * p)
                    cols_sl = slice(PANEL * p, PANEL * p + wr)
                    dead = KP - m
                    denom[cols_sl] += csum[jid, :wr] - dead * e0
                x[rows] = (diag[:m, si].astype(np.float32) * sc
                           ).astype(np.float32).astype(np.float64)
        total += float(np.sum(np.log(denom) - x))
        nrows += cnt
    assert nrows == plan.n2, (nrows, plan.n2)
    return np.float32(total / nrows)


# --------------------------------------------------------------------------
# entry point
# --------------------------------------------------------------------------

def _get_compiled(plan, reps=1):
    key = (plan.n2, plan.d, tuple(plan.counts), reps)
    if key not in _CACHE:
        _CACHE[key] = _build_program(plan, reps=reps)
    return _CACHE[key]


def _prepare(inputs):
    features = np.asarray(inputs["features"])
    labels = np.asarray(inputs["labels"])
    aug_indices = np.asarray(inputs["aug_indices"])

    fn, labs, perm, fn_sorted, labs_sorted = _host_prep(
        features, labels, aug_indices)
    n2, d = fn_sorted.shape
    classes, counts = np.unique(labs_sorted, return_counts=True)
    plan = _Plan(n2, d, counts.tolist())
    cols, lhsTs = _build_host_arrays(plan, fn_sorted)
    ident = np.eye(KP, dtype=np.float32)
    ones = np.ones((KP, 1), dtype=np.float32)
    in_maps = [{"lhsT": lhsTs[core], "cols": cols, "ident": ident,
                "ones": ones} for core in range(NCORES)]
    return plan, cols, lhsTs, in_maps


def kernel(simulate=False, **inputs):
    plan, cols, lhsTs, in_maps = _prepare(inputs)

    if simulate:
        results = _simulate_device(plan, cols, lhsTs)
    else:
        from concourse.bass_utils import run_bass_kernel_spmd

        nc = _get_compiled(plan)
        results = run_bass_kernel_spmd(nc, in_maps,
                                       core_ids=list(range(NCORES))).results

    return np.asarray(_finish(plan, results), dtype=np.float32)


# --------------------------------------------------------------------------
# timing harness (same methodology as the baseline kernel)
# --------------------------------------------------------------------------

def _make_sharded(nc, n_cores):
    import jax
    import concourse.mybir as mybir
    from jax.sharding import Mesh, PartitionSpec
    from jax.experimental.shard_map import shard_map
    from concourse.bass2jax import (_bass_exec_p, install_neuronx_cc_hook,
                                    partition_id_tensor)

    install_neuronx_cc_hook()
    partition_name = (nc.partition_id_tensor.name
                      if nc.partition_id_tensor else None)
    in_names, out_names, out_avals, zero_outs = [], [], [], []
    for alloc in nc.m.functions[0].allocations:
        if not isinstance(alloc, mybir.MemoryLocationSet):
            continue
        name = alloc.memorylocations[0].name
        if alloc.kind == "ExternalInput":
            if name != partition_name:
                in_names.append(name)
        elif alloc.kind == "ExternalOutput":
            out_names.append(name)
            shape = tuple(alloc.tensor_shape)
            dtype = mybir.dt.np(alloc.dtype)
            out_avals.append(jax.core.ShapedArray(shape, dtype))
            zero_outs.append(np.zeros(shape, dtype))
    n_params = len(in_names)
    all_names = in_names + out_names
    if partition_name is not None:
        all_names.append(partition_name)

    def _body(*args):
        operands = list(args)
        if partition_name is not None:
            operands.append(partition_id_tensor())
        outs = _bass_exec_p.bind(
            *operands,
            out_avals=tuple(out_avals),
            in_names=tuple(all_names),
            out_names=tuple(out_names),
            lowering_input_output_aliases=(),
            sim_require_finite=True,
            sim_require_nnan=True,
            nc=nc,
        )
        return tuple(outs)

    devices = jax.devices()[:n_cores]
    mesh = Mesh(np.asarray(devices), ("core",))
    in_specs = (PartitionSpec("core"),) * (n_params + len(out_names))
    out_specs = (PartitionSpec("core"),) * len(out_names)
    donate = tuple(range(n_params, n_params + len(out_names)))
    sharded = jax.jit(
        shard_map(_body, mesh=mesh, in_specs=in_specs, out_specs=out_specs,
                  check_rep=False),
        donate_argnums=donate, keep_unused=True)
    return sharded, in_names, out_names, out_avals, zero_outs, mesh


def _make_runner(nc, in_maps):
    import jax
    from jax.sharding import NamedSharding, PartitionSpec

    sharded, in_names, out_names, out_avals, zero_outs, mesh = _make_sharded(
        nc, NCORES)
    concat_in = [np.concatenate([in_maps[c][n] for c in range(NCORES)], axis=0)
                 for n in in_names]
    sharding = NamedSharding(mesh, PartitionSpec("core"))
    dev_in = [jax.device_put(a, sharding) for a in concat_in]

    def run():
        import time
        zs = [jax.device_put(
            np.zeros((NCORES * z.shape[0], *z.shape[1:]), z.dtype), sharding)
            for z in zero_outs]
        jax.block_until_ready(zs)
        t0 = time.perf_counter()
        out = sharded(*dev_in, *zs)
        jax.block_until_ready(out)
        return time.perf_counter() - t0

    run()  # warmup (compile + first exec)
    return run


def benchmark(loop_reps=129, pairs=10, **inputs):
    """Per-iteration kernel time, cancelling the ~100ms axon dispatch floor:
    interleave timings of a 1-rep NEFF and a `loop_reps`-rep NEFF (HW loop)
    and difference the minima."""
    plan, cols, lhsTs, in_maps = _prepare(inputs)
    run1 = _make_runner(_get_compiled(plan, reps=1), in_maps)
    runR = _make_runner(_get_compiled(plan, reps=loop_reps), in_maps)

    t1s, tRs = [], []
    for _ in range(pairs):
        t1s.append(run1())
        tRs.append(runR())
    m1, mR = min(t1s), min(tRs)
    per_iter = (mR - m1) / (loop_reps - 1)
    print(f"  [bench] min T(1)={m1*1e3:.2f}ms  min T({loop_reps})={mR*1e3:.2f}ms")
    return per_iter * 1e9
